# revision 33
# baseline (speedup 1.0000x reference)
"""Location-dependent 3D conv (AsymConv) on 8 TRN2 NeuronCores.

Math (per output voxel):
    out[b, 0, x, y, z] = sum_{i,j,l in 0..2} Xp[b, x+i, y+j, z+l] * W[x, y, z, (i*3+j)*3+l]
with Xp = edge-padded X by 1 plane on each spatial side.

Strategy (v2, rebuilt from trace analysis of the 60 us baseline):
  - Shard the X spatial axis (96 = 8 cores x 12 planes); host ships halo'd
    shards -> no inter-core communication.
  - Per-core SBUF layout: partition dim = y (96 of 128); free = (b, x, z).
    y-shifts come from 3 host-shipped pre-shifted copies (compute APs must
    start at partition 0/32/64/96); x/z shifts are free-dim offsets.
  - Products X*W run on the DVE in fp16 2x mode (~0.6 us per tap-batch is the
    hard DVE floor; GPSIMD "help" was measured to contend and lose). Taps are
    fused pairwise via small-stride overlapping access patterns: (l=0, l=2)
    z-pairs and, for l==1, (i=0, i=2) x-pairs - 24 pair + 6 single ops.
  - The 27-term accumulation runs on the TensorEngine as identity matmuls
    into PSUM fp32 (x-chunks 5/5/2 planes). Dummy warm-up matmuls during the
    DMA phase ramp the PE clock 1.2 -> 2.4 GHz before real work arrives.
  - DMA: few BIG HWDGE transfers in consumption order on both rings (one
    dma_start per W unit tile, 221/442 kB each, 4.6 kB descriptors);
    separate SBUF tiles per unit avoid false write-after-read serialization
    of the W stream behind DVE reads.
  - l==1 taps need 4B-aligned reads for DVE 2x mode -> ScalarE builds
    z-shifted X copies during the load phase.
  - PSUM -> SBUF fp16 (ScalarE for b0, DVE cast for b1) -> DRAM; host upcasts
    and reassembles.
"""

import os

import numpy as np

# ---- problem constants (hardcoded per harness rules) ----
B = 2
D = 96  # Dx = Dy = Dz
KSZ = 3
NTAP = KSZ**3  # 27
NCORES = 8
XS = D // NCORES  # 12 x-planes per core
XH = XS + 2  # with halo
ZP = D + 2  # padded z

F16 = np.float16
LAST_RESULT = None  # BassKernelResults of the most recent run (for test.py)

_GRAPH_CACHE = {}

N_WARMUP = int(os.environ.get("ASYM_WARMUP", "10"))
# psum x-plane chunks of the 12-plane streams (480/480/192 columns)
CHUNKS = [(0, 5), (5, 5), (10, 2)]

# ---- product unit schedule ----
# kinds: "s0" = first-unit singles (taps (0,0,0) and (0,0,2), issued as two
# singles so the very first op only waits for half of the first W tile),
# "p" = (l0, l2) z-pair for (i, j), "q" = (i0, i2) x-pair for l=1, j,
# "s1" = single (1, j, 1).
# consumption order pairs with the j-availability of the X tiles.
UNITS = [
    ("s0", (0, 0)),
    ("p", (1, 0)),
    ("p", (2, 0)),
    ("p", (0, 1)),
    ("p", (1, 1)),
    ("p", (2, 1)),
    ("p", (0, 2)),
    ("p", (1, 2)),
    ("p", (2, 2)),
    ("q", 0),
    ("q", 1),
    ("q", 2),
    ("s1", 0),
    ("s1", 1),
    ("s1", 2),
]


def _unit_taps(kind, arg):
    if kind == "s0":
        i, j = arg
        return [(i, j, 0), (i, j, 2)]
    if kind == "p":
        i, j = arg
        return [(i, j, 0), (i, j, 2)]
    if kind == "q":
        j = arg
        return [(0, j, 1), (2, j, 1)]
    j = arg
    return [(1, j, 1)]


def _build_graph():
    """Build (and cache) the per-core Bass graph. Same graph for all 8 cores."""
    if "nc" in _GRAPH_CACHE:
        return _GRAPH_CACHE["nc"]

    from concourse import bacc
    from concourse import bass as _bass
    import concourse.mybir as mybir
    from concourse.tile import TileContext

    f16 = mybir.dt.float16
    f32 = mybir.dt.float32

    nc = bacc.Bacc("TRN2", target_bir_lowering=False, debug=False, num_devices=NCORES)

    # y-pre-shifted X copies packed on one tensor: x[y', j, b, x, z]
    x_d = nc.dram_tensor("x", [D, KSZ, B, XH, ZP], f16, kind="ExternalInput")
    # W per unit (y-major), all units concatenated along the free dim:
    # pair units are 2*XS*D = 2304 elems wide, singles XS*D = 1152
    unit_w = [len(_unit_taps(k, a)) * XS * D for (k, a) in UNITS]
    unit_off = np.concatenate([[0], np.cumsum(unit_w)]).tolist()
    w_d = nc.dram_tensor("w", [D, unit_off[-1]], f16, kind="ExternalInput")
    id_d = nc.dram_tensor("ident", [D, D], f16, kind="ExternalInput")
    out_d = nc.dram_tensor("out", [D, B, XS, D], f16, kind="ExternalOutput")

    with TileContext(nc) as tc:
        with (
            tc.tile_pool(name="xp", bufs=1) as xpool,
            tc.tile_pool(name="wp", bufs=1) as wpool,
            tc.tile_pool(name="pp", bufs=6) as ppool,
            tc.tile_pool(name="psp", bufs=1, space="PSUM") as pspool,
        ):
            # ---- static tiles ----
            x_ts = [
                xpool.tile([D, B, XH, ZP], f16, name=f"x_{j}", tag=f"x_{j}")
                for j in range(KSZ)
            ]
            # z-shifted copies for l == 1 (keeps DVE 2x alignment)
            xz_ts = [
                xpool.tile([D, B, XH, ZP], f16, name=f"xz_{j}", tag=f"xz_{j}")
                for j in range(KSZ)
            ]
            w_ts = []
            for ui, (kind, arg) in enumerate(UNITS):
                nt = len(_unit_taps(kind, arg))
                shape = [D, nt, XS, D] if nt == 2 else [D, XS, D]
                w_ts.append(wpool.tile(shape, f16, name=f"w_{ui}", tag=f"w_{ui}"))
            id_t = xpool.tile([D, D], f16, name="id_t", tag="id_t")

            # ---- DMA schedule (HWDGE, both rings, consumption order) ----
            def w_dma(q, ui, half=None):
                o0, o1 = unit_off[ui], unit_off[ui + 1]
                if half is None:
                    q.dma_start(out=w_ts[ui][:], in_=w_d.ap()[:, o0:o1])
                else:
                    h = (o1 - o0) // 2
                    q.dma_start(
                        out=w_ts[ui][:, half : half + 1],
                        in_=w_d.ap()[:, o0 + half * h : o0 + (half + 1) * h],
                    )

            # ACT ring: ident + first W tiles
            nc.scalar.dma_start(out=id_t[:], in_=id_d.ap())
            w_dma(nc.scalar, 0, half=0)
            w_dma(nc.scalar, 0, half=1)
            w_dma(nc.scalar, 1)
            # SP ring: X slices (b-split for earliest first product) + rest of
            # the W stream, interleaved in consumption order
            nc.sync.dma_start(out=x_ts[0][:, 0:1], in_=x_d.ap()[:, 0, 0:1])
            nc.sync.dma_start(out=x_ts[0][:, 1:2], in_=x_d.ap()[:, 0, 1:2])
            w_dma(nc.sync, 2)
            nc.sync.dma_start(out=x_ts[1][:, 0:1], in_=x_d.ap()[:, 1, 0:1])
            nc.sync.dma_start(out=x_ts[1][:, 1:2], in_=x_d.ap()[:, 1, 1:2])
            w_dma(nc.sync, 3)
            nc.sync.dma_start(out=x_ts[2][:, 0:1], in_=x_d.ap()[:, 2, 0:1])
            nc.sync.dma_start(out=x_ts[2][:, 1:2], in_=x_d.ap()[:, 2, 1:2])
            for ui in range(4, 9):
                w_dma(nc.sync, ui)

            # ---- PE warm-up during the DMA phase (no DMA dependency: uses
            # the memset dummy as both weights and rhs) ----
            if N_WARMUP:
                dummy = ppool.tile([D, 480], f16, name="dummy", tag="warm_rhs", bufs=1)
                nc.vector.memset(dummy[:], 0.0)
                ps_w = pspool.tile([D, 480], f32, name="ps_warm", tag="ps_warm")
                for _ in range(N_WARMUP):
                    nc.tensor.matmul(
                        ps_w[:], dummy[:, 0:D], dummy[:], start=True, stop=True
                    )

            # ---- ScalarE: z-shifted copies (for l == 1 units), interleaved
            # with the late l=1 W dispatches so neither blocks the other ----
            for j in range(KSZ):
                nc.scalar.copy(
                    out=xz_ts[j][:, :, :, 0 : ZP - 1], in_=x_ts[j][:, :, :, 1:ZP]
                )
                w_dma(nc.scalar, 9 + j)
            for ui in range(12, len(UNITS)):
                w_dma(nc.scalar, ui)

            # ---- product + accumulate schedule ----
            psums = {
                (b, ci): pspool.tile(
                    [D, nx, D], f32, name=f"ps_{b}_{ci}", tag=f"ps_{b}_{ci}"
                )
                for b in range(B)
                for ci, (x0, nx) in enumerate(CHUNKS)
            }

            def zpair_ap(j, b, i):
                """[D, 2, XS, D] view of x_ts[j]: overlapping z-windows l=0,2."""
                base = x_ts[j][:, b, i : i + XS, 0:D]
                ap = list(base.ap)
                return _bass.AP(
                    base.tensor, base.offset, [ap[0], [2, 2], ap[1], ap[2]]
                )

            def xpair_ap(j, b):
                """[D, 2, XS, D] view of xz_ts[j]: overlapping x-windows i=0,2."""
                base = xz_ts[j][:, b, 0:XS, 0:D]
                ap = list(base.ap)
                return _bass.AP(
                    base.tensor, base.offset, [ap[0], [2 * ap[1][0], 2], ap[1], ap[2]]
                )

            # per (b, chunk) accumulation counters for start/stop flags
            seen = {(b, ci): 0 for b in range(B) for ci in range(len(CHUNKS))}

            def consume(prod, b, nt):
                """PE: accumulate nt tap-streams of a product tile into psums.
                Chunk-inner order: consecutive matmuls hit different PSUM banks
                (same-bank back-to-back stalls the accumulate pipeline)."""
                for t in range(nt):
                    for ci, (x0, nx) in enumerate(CHUNKS):
                        s = seen[(b, ci)]
                        rhs = (
                            prod[:, t, x0 : x0 + nx, :]
                            if nt == 2
                            else prod[:, x0 : x0 + nx, :]
                        )
                        nc.tensor.matmul(
                            psums[(b, ci)][:],
                            id_t[:],
                            rhs,
                            start=(s == 0),
                            stop=(s == NTAP - 1),
                        )
                        seen[(b, ci)] = s + 1

            for ui, (kind, arg) in enumerate(UNITS):
                for b in range(B):
                    if kind == "s0":
                        # two singles: first waits only on half the W tile
                        i, j = arg
                        for s, l in enumerate((0, 2)):
                            prod = ppool.tile(
                                [D, XS, D], f16, name="prod1", tag="prod1", bufs=4
                            )
                            nc.vector.tensor_mul(
                                out=prod[:],
                                in0=x_ts[j][:, b, i : i + XS, l : l + D],
                                in1=w_ts[ui][:, s],
                            )
                            consume(prod, b, 1)
                        continue
                    if kind == "p":
                        i, j = arg
                        src = zpair_ap(j, b, i)
                    elif kind == "q":
                        j = arg
                        src = xpair_ap(j, b)
                    else:
                        j = arg
                        src = xz_ts[j][:, b, 1 : 1 + XS, 0:D]
                    nt = len(_unit_taps(kind, arg))
                    if nt == 2:
                        prod = ppool.tile(
                            [D, 2, XS, D], f16, name="prod2", tag="prod2", bufs=5
                        )
                    else:
                        prod = ppool.tile(
                            [D, XS, D], f16, name="prod1", tag="prod1", bufs=4
                        )
                    nc.vector.tensor_mul(out=prod[:], in0=src, in1=w_ts[ui][:])
                    consume(prod, b, nt)

            # ---- evacuate PSUM -> SBUF f16 -> DRAM ----
            for b in range(B):
                for ci, (x0, nx) in enumerate(CHUNKS):
                    outsb = ppool.tile(
                        [D, nx, D], f16, name="outsb", tag=f"outsb_{b}_{ci}", bufs=1
                    )
                    if b == 0:
                        nc.scalar.copy(out=outsb[:], in_=psums[(b, ci)][:])
                    else:
                        nc.vector.tensor_copy(out=outsb[:], in_=psums[(b, ci)][:])
                    q = nc.sync if ci % 2 == 0 else nc.scalar
                    q.dma_start(
                        out=out_d.ap()[:, b, x0 : x0 + nx, :],
                        in_=outsb[:],
                    )

    nc.compile()
    _GRAPH_CACHE["nc"] = nc
    return nc


def make_in_maps(X, W):
    """Host-side shard prep. X [2,1,96,96,96] f32, W [1,1,96,96,96,27] f32."""
    X = np.asarray(X)
    W = np.asarray(W)
    Xs = X.reshape(B, D, D, D)
    # edge padding on all three spatial dims
    Xp = np.pad(Xs, ((0, 0), (1, 1), (1, 1), (1, 1)), mode="edge")
    # -> [y, b, x, z]
    Xt = np.ascontiguousarray(np.transpose(Xp, (2, 0, 1, 3))).astype(F16)
    W00 = W.reshape(D, D, D, NTAP)  # [x, y, z, tap]
    ident = np.eye(D, dtype=F16)

    in_maps = []
    for m in range(NCORES):
        xs_full = Xt[:, :, m * XS : m * XS + XH, :]  # [98, 2, 14, 98]
        im = {"ident": ident}
        # x[y, j, b, x, z]
        im["x"] = np.ascontiguousarray(
            np.stack([xs_full[j : j + D] for j in range(KSZ)], axis=1)
        )  # [96, 3, 2, 14, 98]
        wm = W00[m * XS : (m + 1) * XS]  # [12, 96, 96, 27]
        wmt = np.transpose(wm, (1, 0, 2, 3))  # [y, x, z, tap]
        blocks = []
        for kind, arg in UNITS:
            taps = _unit_taps(kind, arg)
            idxs = [(i * KSZ + j) * KSZ + l for (i, j, l) in taps]
            blk = wmt[:, :, :, idxs]  # [y, x, z, nt]
            if len(idxs) == 2:
                wt = np.transpose(blk, (0, 3, 1, 2))  # [y, nt, x, z]
            else:
                wt = blk[:, :, :, 0]  # [y, x, z]
            blocks.append(wt.reshape(D, -1))
        im["w"] = np.ascontiguousarray(np.concatenate(blocks, axis=1)).astype(F16)
        in_maps.append(im)
    return in_maps


def kernel(X, W):
    global LAST_RESULT
    from concourse.bass_utils import run_bass_kernel_spmd

    nc = _build_graph()
    in_maps = make_in_maps(X, W)
    trace = bool(int(os.environ.get("ASYM_TRACE", "0")))
    res = run_bass_kernel_spmd(
        nc, in_maps, core_ids=list(range(NCORES)), trace=trace
    )
    LAST_RESULT = res

    out = np.empty((B, 1, D, D, D), dtype=np.float32)
    for m in range(NCORES):
        r = res.results[m]["out"].astype(np.float32)  # [y, b, x, z]
        out[:, 0, m * XS : (m + 1) * XS, :, :] = np.transpose(r, (1, 2, 0, 3))
    return out


# revision 34
# speedup vs baseline: 1.1415x; 1.1415x over previous
"""Location-dependent 3D conv (AsymConv) on 8 TRN2 NeuronCores.

Math (per output voxel):
    out[b, 0, x, y, z] = sum_{i,j,l in 0..2} Xp[b, x+i, y+j, z+l] * W[x, y, z, (i*3+j)*3+l]
with Xp = edge-padded X by 1 plane on each spatial side.

Strategy (v2, rebuilt from trace analysis of the 60 us baseline):
  - Shard the X spatial axis (96 = 8 cores x 12 planes); host ships halo'd
    shards -> no inter-core communication.
  - Per-core SBUF layout: partition dim = y (96 of 128); free = (b, x, z).
    y-shifts come from 3 host-shipped pre-shifted copies (compute APs must
    start at partition 0/32/64/96); x/z shifts are free-dim offsets.
  - Products X*W run on the DVE in fp16 2x mode (~0.6 us per tap-batch is the
    hard DVE floor; GPSIMD "help" was measured to contend and lose). Taps are
    fused pairwise via small-stride overlapping access patterns: (l=0, l=2)
    z-pairs and, for l==1, (i=0, i=2) x-pairs - 24 pair + 6 single ops.
  - The 27-term accumulation runs on the TensorEngine as identity matmuls
    into PSUM fp32 (x-chunks 5/5/2 planes). Dummy warm-up matmuls during the
    DMA phase ramp the PE clock 1.2 -> 2.4 GHz before real work arrives.
  - DMA: few BIG HWDGE transfers in consumption order on both rings (one
    dma_start per W unit tile, 221/442 kB each, 4.6 kB descriptors);
    separate SBUF tiles per unit avoid false write-after-read serialization
    of the W stream behind DVE reads.
  - l==1 taps need 4B-aligned reads for DVE 2x mode -> ScalarE builds
    z-shifted X copies during the load phase.
  - PSUM -> SBUF fp16 (ScalarE for b0, DVE cast for b1) -> DRAM; host upcasts
    and reassembles.
"""

import os

import numpy as np

# ---- problem constants (hardcoded per harness rules) ----
B = 2
D = 96  # Dx = Dy = Dz
KSZ = 3
NTAP = KSZ**3  # 27
NCORES = 8
XS = D // NCORES  # 12 x-planes per core
XH = XS + 2  # with halo
ZP = D + 2  # padded z

F16 = np.float16
LAST_RESULT = None  # BassKernelResults of the most recent run (for test.py)

_GRAPH_CACHE = {}

N_WARMUP = int(os.environ.get("ASYM_WARMUP", "0"))
# psum x-plane chunks of the 12-plane streams (480/480/192 columns)
CHUNKS = [(0, 5), (5, 5), (10, 2)]

# ---- product unit schedule ----
# kinds: "s0" = first-unit singles (taps (0,0,0) and (0,0,2), issued as two
# singles so the very first op only waits for half of the first W tile),
# "p" = (l0, l2) z-pair for (i, j), "q" = (i0, i2) x-pair for l=1, j,
# "s1" = single (1, j, 1).
# consumption order pairs with the j-availability of the X tiles.
UNITS = [
    ("s0", (0, 0)),
    ("p", (1, 0)),
    ("p", (2, 0)),
    ("p", (0, 1)),
    ("p", (1, 1)),
    ("p", (2, 1)),
    ("p", (0, 2)),
    ("p", (1, 2)),
    ("p", (2, 2)),
    ("q", 0),
    ("q", 1),
    ("q", 2),
    ("s1", 0),
    ("s1", 1),
    ("s1", 2),
]


def _unit_taps(kind, arg):
    if kind == "s0":
        i, j = arg
        return [(i, j, 0), (i, j, 2)]
    if kind == "p":
        i, j = arg
        return [(i, j, 0), (i, j, 2)]
    if kind == "q":
        j = arg
        return [(0, j, 1), (2, j, 1)]
    j = arg
    return [(1, j, 1)]


def _build_graph():
    """Build (and cache) the per-core Bass graph. Same graph for all 8 cores."""
    if "nc" in _GRAPH_CACHE:
        return _GRAPH_CACHE["nc"]

    from concourse import bacc
    from concourse import bass as _bass
    import concourse.mybir as mybir
    from concourse.tile import TileContext

    f16 = mybir.dt.float16
    f32 = mybir.dt.float32

    nc = bacc.Bacc("TRN2", target_bir_lowering=False, debug=False, num_devices=NCORES)

    # y-pre-shifted X copies packed on one tensor: x[y', j, b, x, z]
    x_d = nc.dram_tensor("x", [D, KSZ, B, XH, ZP], f16, kind="ExternalInput")
    # W per unit (y-major), all units concatenated along the free dim:
    # pair units are 2*XS*D = 2304 elems wide, singles XS*D = 1152
    unit_w = [len(_unit_taps(k, a)) * XS * D for (k, a) in UNITS]
    unit_off = np.concatenate([[0], np.cumsum(unit_w)]).tolist()
    w_d = nc.dram_tensor("w", [D, unit_off[-1]], f16, kind="ExternalInput")
    id_d = nc.dram_tensor("ident", [D, D], f16, kind="ExternalInput")
    out_d = nc.dram_tensor("out", [D, B, XS, D], f16, kind="ExternalOutput")

    with TileContext(nc) as tc:
        with (
            tc.tile_pool(name="xp", bufs=1) as xpool,
            tc.tile_pool(name="wp", bufs=1) as wpool,
            tc.tile_pool(name="pp", bufs=6) as ppool,
            tc.tile_pool(name="psp", bufs=1, space="PSUM") as pspool,
        ):
            # ---- static tiles ----
            x_ts = [
                xpool.tile([D, B, XH, ZP], f16, name=f"x_{j}", tag=f"x_{j}")
                for j in range(KSZ)
            ]
            # z-shifted copies for l == 1 (keeps DVE 2x alignment)
            xz_ts = [
                xpool.tile([D, B, XH, ZP], f16, name=f"xz_{j}", tag=f"xz_{j}")
                for j in range(KSZ)
            ]
            w_ts = []
            for ui, (kind, arg) in enumerate(UNITS):
                nt = len(_unit_taps(kind, arg))
                shape = [D, nt, XS, D] if nt == 2 else [D, XS, D]
                w_ts.append(wpool.tile(shape, f16, name=f"w_{ui}", tag=f"w_{ui}"))
            id_t = xpool.tile([D, D], f16, name="id_t", tag="id_t")

            # ---- DMA schedule (HWDGE, both rings, consumption order) ----
            def w_dma(q, ui, half=None):
                o0, o1 = unit_off[ui], unit_off[ui + 1]
                if half is None:
                    q.dma_start(out=w_ts[ui][:], in_=w_d.ap()[:, o0:o1])
                else:
                    h = (o1 - o0) // 2
                    q.dma_start(
                        out=w_ts[ui][:, half : half + 1],
                        in_=w_d.ap()[:, o0 + half * h : o0 + (half + 1) * h],
                    )

            # ACT ring: ident + first W tiles
            nc.scalar.dma_start(out=id_t[:], in_=id_d.ap())
            w_dma(nc.scalar, 0, half=0)
            w_dma(nc.scalar, 0, half=1)
            w_dma(nc.scalar, 1)
            # SP ring: X slices (b-split for earliest first product) + rest of
            # the W stream, interleaved in consumption order
            nc.sync.dma_start(out=x_ts[0][:, 0:1], in_=x_d.ap()[:, 0, 0:1])
            nc.sync.dma_start(out=x_ts[0][:, 1:2], in_=x_d.ap()[:, 0, 1:2])
            w_dma(nc.sync, 2)
            nc.sync.dma_start(out=x_ts[1][:, 0:1], in_=x_d.ap()[:, 1, 0:1])
            nc.sync.dma_start(out=x_ts[1][:, 1:2], in_=x_d.ap()[:, 1, 1:2])
            w_dma(nc.sync, 3)
            nc.sync.dma_start(out=x_ts[2][:, 0:1], in_=x_d.ap()[:, 2, 0:1])
            nc.sync.dma_start(out=x_ts[2][:, 1:2], in_=x_d.ap()[:, 2, 1:2])
            for ui in range(4, 9):
                w_dma(nc.sync, ui)

            # ---- PE warm-up during the DMA phase (no DMA dependency: uses
            # the memset dummy as both weights and rhs) ----
            if N_WARMUP:
                dummy = ppool.tile([D, 480], f16, name="dummy", tag="warm_rhs", bufs=1)
                nc.vector.memset(dummy[:], 0.0)
                ps_w = pspool.tile([D, 480], f32, name="ps_warm", tag="ps_warm")
                for _ in range(N_WARMUP):
                    nc.tensor.matmul(
                        ps_w[:], dummy[:, 0:D], dummy[:], start=True, stop=True
                    )

            # ---- ScalarE: z-shifted copies (for l == 1 units), interleaved
            # with the late l=1 W dispatches so neither blocks the other ----
            for j in range(KSZ):
                nc.scalar.copy(
                    out=xz_ts[j][:, :, :, 0 : ZP - 1], in_=x_ts[j][:, :, :, 1:ZP]
                )
                w_dma(nc.scalar, 9 + j)
            for ui in range(12, len(UNITS)):
                w_dma(nc.scalar, ui)

            # ---- product + accumulate schedule ----
            psums = {
                (b, ci): pspool.tile(
                    [D, nx, D], f32, name=f"ps_{b}_{ci}", tag=f"ps_{b}_{ci}"
                )
                for b in range(B)
                for ci, (x0, nx) in enumerate(CHUNKS)
            }

            def zpair_ap(j, b, i):
                """[D, 2, XS, D] view of x_ts[j]: overlapping z-windows l=0,2."""
                base = x_ts[j][:, b, i : i + XS, 0:D]
                ap = list(base.ap)
                return _bass.AP(
                    base.tensor, base.offset, [ap[0], [2, 2], ap[1], ap[2]]
                )

            def xpair_ap(j, b):
                """[D, 2, XS, D] view of xz_ts[j]: overlapping x-windows i=0,2."""
                base = xz_ts[j][:, b, 0:XS, 0:D]
                ap = list(base.ap)
                return _bass.AP(
                    base.tensor, base.offset, [ap[0], [2 * ap[1][0], 2], ap[1], ap[2]]
                )

            # per (b, chunk) accumulation counters for start/stop flags
            seen = {(b, ci): 0 for b in range(B) for ci in range(len(CHUNKS))}

            def consume(prod, b, nt):
                """PE: accumulate nt tap-streams of a product tile into psums.
                Chunk-inner order: consecutive matmuls hit different PSUM banks
                (same-bank back-to-back stalls the accumulate pipeline)."""
                for t in range(nt):
                    for ci, (x0, nx) in enumerate(CHUNKS):
                        s = seen[(b, ci)]
                        rhs = (
                            prod[:, t, x0 : x0 + nx, :]
                            if nt == 2
                            else prod[:, x0 : x0 + nx, :]
                        )
                        nc.tensor.matmul(
                            psums[(b, ci)][:],
                            id_t[:],
                            rhs,
                            start=(s == 0),
                            stop=(s == NTAP - 1),
                        )
                        seen[(b, ci)] = s + 1

            for ui, (kind, arg) in enumerate(UNITS):
                for b in range(B):
                    if kind == "s0":
                        # two singles: first waits only on half the W tile
                        i, j = arg
                        for s, l in enumerate((0, 2)):
                            prod = ppool.tile(
                                [D, XS, D], f16, name="prod1", tag="prod1", bufs=4
                            )
                            nc.vector.tensor_mul(
                                out=prod[:],
                                in0=x_ts[j][:, b, i : i + XS, l : l + D],
                                in1=w_ts[ui][:, s],
                            )
                            consume(prod, b, 1)
                        continue
                    if kind == "p":
                        i, j = arg
                        src = zpair_ap(j, b, i)
                    elif kind == "q":
                        j = arg
                        src = xpair_ap(j, b)
                    else:
                        j = arg
                        src = xz_ts[j][:, b, 1 : 1 + XS, 0:D]
                    nt = len(_unit_taps(kind, arg))
                    if nt == 2:
                        prod = ppool.tile(
                            [D, 2, XS, D], f16, name="prod2", tag="prod2", bufs=5
                        )
                    else:
                        prod = ppool.tile(
                            [D, XS, D], f16, name="prod1", tag="prod1", bufs=4
                        )
                    nc.vector.tensor_mul(out=prod[:], in0=src, in1=w_ts[ui][:])
                    consume(prod, b, nt)

            # ---- evacuate PSUM -> SBUF f16 -> DRAM ----
            for b in range(B):
                for ci, (x0, nx) in enumerate(CHUNKS):
                    outsb = ppool.tile(
                        [D, nx, D], f16, name="outsb", tag=f"outsb_{b}_{ci}", bufs=1
                    )
                    if b == 0:
                        nc.scalar.copy(out=outsb[:], in_=psums[(b, ci)][:])
                    else:
                        nc.vector.tensor_copy(out=outsb[:], in_=psums[(b, ci)][:])
                    q = nc.sync if ci % 2 == 0 else nc.scalar
                    q.dma_start(
                        out=out_d.ap()[:, b, x0 : x0 + nx, :],
                        in_=outsb[:],
                    )

    nc.compile()
    _GRAPH_CACHE["nc"] = nc
    return nc


def make_in_maps(X, W):
    """Host-side shard prep. X [2,1,96,96,96] f32, W [1,1,96,96,96,27] f32."""
    X = np.asarray(X)
    W = np.asarray(W)
    Xs = X.reshape(B, D, D, D)
    # edge padding on all three spatial dims
    Xp = np.pad(Xs, ((0, 0), (1, 1), (1, 1), (1, 1)), mode="edge")
    # -> [y, b, x, z]
    Xt = np.ascontiguousarray(np.transpose(Xp, (2, 0, 1, 3))).astype(F16)
    W00 = W.reshape(D, D, D, NTAP)  # [x, y, z, tap]
    ident = np.eye(D, dtype=F16)

    in_maps = []
    for m in range(NCORES):
        xs_full = Xt[:, :, m * XS : m * XS + XH, :]  # [98, 2, 14, 98]
        im = {"ident": ident}
        # x[y, j, b, x, z]
        im["x"] = np.ascontiguousarray(
            np.stack([xs_full[j : j + D] for j in range(KSZ)], axis=1)
        )  # [96, 3, 2, 14, 98]
        wm = W00[m * XS : (m + 1) * XS]  # [12, 96, 96, 27]
        wmt = np.transpose(wm, (1, 0, 2, 3))  # [y, x, z, tap]
        blocks = []
        for kind, arg in UNITS:
            taps = _unit_taps(kind, arg)
            idxs = [(i * KSZ + j) * KSZ + l for (i, j, l) in taps]
            blk = wmt[:, :, :, idxs]  # [y, x, z, nt]
            if len(idxs) == 2:
                wt = np.transpose(blk, (0, 3, 1, 2))  # [y, nt, x, z]
            else:
                wt = blk[:, :, :, 0]  # [y, x, z]
            blocks.append(wt.reshape(D, -1))
        im["w"] = np.ascontiguousarray(np.concatenate(blocks, axis=1)).astype(F16)
        in_maps.append(im)
    return in_maps


def kernel(X, W):
    global LAST_RESULT
    from concourse.bass_utils import run_bass_kernel_spmd

    nc = _build_graph()
    in_maps = make_in_maps(X, W)
    trace = bool(int(os.environ.get("ASYM_TRACE", "0")))
    res = run_bass_kernel_spmd(
        nc, in_maps, core_ids=list(range(NCORES)), trace=trace
    )
    LAST_RESULT = res

    out = np.empty((B, 1, D, D, D), dtype=np.float32)
    for m in range(NCORES):
        r = res.results[m]["out"].astype(np.float32)  # [y, b, x, z]
        out[:, 0, m * XS : (m + 1) * XS, :, :] = np.transpose(r, (1, 2, 0, 3))
    return out
